# revision 6
# baseline (speedup 1.0000x reference)
"""BitConv2dInfer on 8 Trainium2 NeuronCores.

Reference computation (per full input):
    x = clip(x, -1, 1)                       # x [32, 256, 56, 56] f32
    y = conv2d(x, w_q, pad=1)                # w_q [256, 256, 3, 3] ternary
    y = y * s + bias                         # per-out-channel affine
Sharding: data-parallel over batch — each of the 8 cores gets 4 images and
the full (tiny) weights; outputs concatenate over batch with no comms.

Device kernel (per core, per image) — fp8 DoubleRow variant:
  - DMA x[n] in as 2 CIN tiles of [128, 56, 56] f32 (row-chunked for the
    first image so the PE can start before the full image lands)
  - x is split per element into hi = fp8(clip(x)) and lo = fp8(clip(x) - hi)
    (e4m3 has a 4-bit significand, so hi+lo carries ~8 significand bits —
    bf16-grade accuracy) via: V clamp f32->bf16, G copy bf16->fp8 into a
    zero-bordered [128, 2, 58, 58] hi tile, V subtract (bf16 - fp8) -> fp8
    into the matching lo tile.
  - conv as 18 accumulated DoubleRow PE matmuls per (cout_tile, 8-row chunk):
    per tap t: psum += sum_ci w[t,ci].T @ hi_win[ci]; psum += same w @ lo_win
    Each DoubleRow matmul contracts both cin tiles at once (lhsT [128,2,128]
    fp8, rhs [128, 2, 8, 56] fp8) at 0.5 PE cycles per output row — 2x the
    bf16 rate.
  - scalar-engine activation evacuates PSUM with per-partition scale+bias
  - DMA f32 result tiles back out (finely chunked for the last image so the
    tail drains early)

The PE clock gate (HAM) starts at 1.2 GHz and only reaches 2.4 GHz after
~3.4us of sustained activity, so the kernel front-runs dummy matmuls on a
zeroed tile while the first input chunks are in flight.

Weights are host-side transposed to lhsT layout [128 cin, co, (tap, ci), cout]
and cast to fp8e4m3 (exact for ternary values).
"""

import sys

sys.path.insert(0, "/opt/trn_rl_repo")

import ml_dtypes
import numpy as np

import concourse.bass as bass  # noqa: F401  (registers engines)
import concourse.mybir as mybir
import concourse.tile as tile
from concourse import bacc
from concourse.bass_utils import run_bass_kernel_spmd

N, CIN, COUT, H, W = 32, 256, 256, 56, 56
NCORES = 8
NB = N // NCORES          # images per core
HP, WP = H + 2, W + 2     # padded spatial
RG = 8                    # output rows per PSUM chunk (8*56=448 <= 512 f32/bank)
NCH = H // RG             # chunks per image
NCI = CIN // 128          # cin tiles
NCO = COUT // 128         # cout tiles
NTAP = 9
# First-image input chunk schedule, (ci, engine, row0, nrows) in issue order.
# Chunks are sized/placed so each ring (gpsimd / scalar, first byte ~10.5us)
# delivers every chunk before its (now ~1.7us-per-group) deadline. A 9-row
# first chunk makes matmul group 0 depend on chunk 0 alone.
N0_CHUNKS = [
    ("g", 0, 0, 9), ("s", 1, 0, 9),
    ("g", 0, 9, 8), ("s", 1, 9, 8),
    ("g", 0, 17, 16), ("s", 1, 17, 8),
    ("s", 1, 25, 8),
    ("g", 0, 33, 12), ("s", 1, 33, 12),
    ("g", 0, 45, 11), ("s", 1, 45, 11),
]
N_WARM_MM = 17            # dummy matmuls to lift the HAM clock gate

F8 = mybir.dt.float8e4
DR = mybir.MatmulPerfMode.DoubleRow

_compiled = {}


def _build():
    nc = bacc.Bacc("TRN2", target_bir_lowering=False, debug=False)
    f32, bf16 = mybir.dt.float32, mybir.dt.bfloat16
    x_d = nc.dram_tensor("x", [NB, CIN, H, W], f32, kind="ExternalInput").ap()
    w_d = nc.dram_tensor(
        "w", [128, NCO, NTAP * NCI, 128], F8, kind="ExternalInput"
    ).ap()
    sb_d = nc.dram_tensor("sb", [128, 2 * NCO], f32, kind="ExternalInput").ap()
    o_d = nc.dram_tensor("out", [NB, COUT, H, W], f32, kind="ExternalOutput").ap()

    clamp = dict(op0=mybir.AluOpType.max, op1=mybir.AluOpType.min)

    with tile.TileContext(nc) as tc:
        with (
            tc.tile_pool(name="const", bufs=1) as cpool,
            tc.tile_pool(name="xs", bufs=4) as xspool,
            tc.tile_pool(name="xsc", bufs=3) as xscpool,
            tc.tile_pool(name="xc", bufs=3) as xcpool,
            tc.tile_pool(name="xpad", bufs=2) as xppool,
            tc.tile_pool(name="osb", bufs=3) as opool,
            tc.tile_pool(name="ps", bufs=6, space="PSUM") as pspool,
            tc.tile_pool(name="warmps", bufs=1, space="PSUM") as wpspool,
        ):
            w_sb = cpool.tile([128, NCO, NTAP * NCI, 128], F8, tag="w")
            sb_sb = cpool.tile([128, 2 * NCO], f32, tag="sb")

            # HAM pre-warm (memset on gpsimd so the vector engine's queue
            # stays clear for the border memsets + clamps that gate the
            # first real matmul group).
            warm = cpool.tile([128, RG * W], bf16, tag="warm")
            nc.gpsimd.memset(warm[:], 0.0)
            warm_ps = wpspool.tile([128, RG * W], f32, tag="warmps")
            for _ in range(N_WARM_MM):
                nc.tensor.matmul(
                    out=warm_ps[:], lhsT=warm[:, 0:128], rhs=warm[:],
                    start=True, stop=True,
                )

            def border_memsets(t):
                for ci in range(NCI):
                    nc.vector.memset(t[:, ci, 0:1, :], 0.0)
                    nc.vector.memset(t[:, ci, HP - 1:HP, :], 0.0)
                    nc.vector.memset(t[:, ci, 1:HP - 1, 0:1], 0.0)
                    nc.vector.memset(t[:, ci, 1:HP - 1, WP - 1:WP], 0.0)

            # hi/lo split of a clamped f32 chunk into the padded fp8 tiles.
            # The gpsimd re-clamp is a no-op arithmetically; it is the
            # bf16 -> fp8 cast (gpsimd has no plain copy).
            def split_chunk(xs_f32, xph, xpl, xc_bf, ci, r0, nr):
                nc.vector.tensor_scalar(
                    xc_bf[:, r0:r0 + nr], xs_f32, -1.0, 1.0, **clamp,
                )
                nc.gpsimd.tensor_scalar(
                    xph[:, ci, r0 + 1:r0 + nr + 1, 1:W + 1],
                    xc_bf[:, r0:r0 + nr], -1.0, 1.0, **clamp,
                )
                nc.vector.tensor_tensor(
                    out=xpl[:, ci, r0 + 1:r0 + nr + 1, 1:W + 1],
                    in0=xc_bf[:, r0:r0 + nr],
                    in1=xph[:, ci, r0 + 1:r0 + nr + 1, 1:W + 1],
                    op=mybir.AluOpType.subtract,
                )

            # First image, row-chunked. Critical set for the first matmul
            # group: w piece (taps 0-2 of co=0) + 9 input rows of both ci.
            n0_xph = xppool.tile([128, NCI, HP, WP], F8, tag="xph")
            n0_xpl = xppool.tile([128, NCI, HP, WP], F8, tag="xpl")
            border_memsets(n0_xph)
            border_memsets(n0_xpl)
            n0_xc = [
                xcpool.tile([128, H, W], bf16, tag=f"xc{ci}", name=f"n0xc{ci}")
                for ci in range(NCI)
            ]
            n0_stage = []
            for k, (eng_key, ci, r0, nr) in enumerate(N0_CHUNKS):
                eng = nc.gpsimd if eng_key == "g" else nc.scalar
                xs = xscpool.tile([128, 16, W], f32, tag=f"xsc{ci}")
                eng.dma_start(
                    out=xs[:, 0:nr],
                    in_=x_d[0, ci * 128:(ci + 1) * 128, r0:r0 + nr],
                )
                n0_stage.append((r0, nr, ci, xs))
                if k == 1:
                    # First w piece right after the critical x chunks.
                    nc.sync.dma_start(out=w_sb[:, 0, 0:6], in_=w_d[:, 0, 0:6])
                    nc.sync.dma_start(out=w_sb[:, 0, 6:18], in_=w_d[:, 0, 6:18])
            # Non-critical constants last: they'd steal queue bandwidth from
            # the chunk stream the PE is about to wait on.
            nc.sync.dma_start(out=w_sb[:, 1], in_=w_d[:, 1])
            nc.gpsimd.dma_start(out=sb_sb[:], in_=sb_d)
            for r0, nr, ci, xs in n0_stage:
                split_chunk(xs[:, 0:nr], n0_xph, n0_xpl, n0_xc[ci], ci, r0, nr)

            # Full-image input DMA, issued one image ahead of its use so the
            # issue (and the transfer) runs behind the previous image's PE
            # work instead of serializing after its PSUM evictions.
            def issue_input(n):
                tiles = []
                for ci, eng in ((0, nc.gpsimd), (1, nc.scalar)):
                    xs = xspool.tile([128, H, W], f32, tag="xs")
                    eng.dma_start(out=xs[:], in_=x_d[n, ci * 128:(ci + 1) * 128])
                    tiles.append(xs)
                return tiles

            pending = None
            for n in range(NB):
                if n == 0:
                    xph, xpl = n0_xph, n0_xpl
                else:
                    xph = xppool.tile([128, NCI, HP, WP], F8, tag="xph")
                    xpl = xppool.tile([128, NCI, HP, WP], F8, tag="xpl")
                    border_memsets(xph)
                    border_memsets(xpl)
                    for ci in range(NCI):
                        xc = xcpool.tile([128, H, W], bf16, tag=f"xc{ci}")
                        split_chunk(pending[ci][:], xph, xpl, xc, ci, 0, H)
                if n + 1 < NB:
                    pending = issue_input(n + 1)
                for co in range(NCO):
                    last_tile = n == NB - 1 and co == NCO - 1
                    osb = opool.tile([128, H, W], f32, tag="osb")
                    if last_tile:
                        # Final output tile: taper the last groups so the
                        # closing ACT + DMA are small and the tail drains fast.
                        groups = [(c * RG, RG) for c in range(NCH - 1)]
                        groups += [(H - 8, 4), (H - 4, 2), (H - 2, 2)]
                    else:
                        groups = [(c * RG, RG) for c in range(NCH)]
                    for g0, gn in groups:
                        ps = pspool.tile([128, RG, W], f32, tag="ps")
                        for t in range(NTAP):
                            kh, kw = divmod(t, 3)
                            lhsT = w_sb[:, co, t * NCI:(t + 1) * NCI]
                            for xp, start, stop in (
                                (xph, t == 0, False),
                                (xpl, False, t == NTAP - 1),
                            ):
                                nc.tensor.matmul(
                                    out=ps[:, 0:gn],
                                    lhsT=lhsT,
                                    rhs=xp[:, :, g0 + kh:g0 + kh + gn, kw:kw + W],
                                    start=start, stop=stop,
                                    perf_mode=DR,
                                )
                        nc.scalar.activation(
                            out=osb[:, g0:g0 + gn, :], in_=ps[:, 0:gn],
                            func=mybir.ActivationFunctionType.Identity,
                            bias=sb_sb[:, NCO + co:NCO + co + 1],
                            scale=sb_sb[:, co:co + 1],
                        )
                        if last_tile:
                            nc.sync.dma_start(
                                out=o_d[n, co * 128:(co + 1) * 128, g0:g0 + gn],
                                in_=osb[:, g0:g0 + gn],
                            )
                    dst = o_d[n, co * 128:(co + 1) * 128]
                    if not last_tile:
                        nc.sync.dma_start(out=dst[:, 0:32], in_=osb[:, 0:32])
                        nc.sync.dma_start(out=dst[:, 32:H], in_=osb[:, 32:H])

    nc.compile()
    return nc


def _prep_weights(w_q, s, bias):
    # lhsT layout: [cin_k (128 partitions), co, (tap, ci), cout_j] so that
    # w_t[k, co, t*2+ci, j] = w_q[co*128 + j, ci*128 + k, kh, kw]
    w_t = (
        w_q.astype(np.float32)
        .transpose(2, 3, 1, 0)                 # [kh, kw, CIN, COUT]
        .reshape(NTAP, NCI, 128, NCO, 128)     # [tap, ci, k, co, j]
        .transpose(2, 3, 0, 1, 4)              # [k, co, tap, ci, j]
        .reshape(128, NCO, NTAP * NCI, 128)
        .astype(ml_dtypes.float8_e4m3)
    )
    sb_t = np.concatenate(
        [
            np.ascontiguousarray(s.reshape(NCO, 128).T.astype(np.float32)),
            np.ascontiguousarray(bias.reshape(NCO, 128).T.astype(np.float32)),
        ],
        axis=1,
    )
    return w_t, np.ascontiguousarray(sb_t)


def kernel(x, w_q, s, bias):
    if "nc" not in _compiled:
        _compiled["nc"] = _build()
    nc = _compiled["nc"]

    w_t, sb_t = _prep_weights(w_q, s, bias)
    x = np.ascontiguousarray(x, dtype=np.float32)
    core_ids = list(range(NCORES))
    in_maps = [
        {"x": x[i * NB:(i + 1) * NB], "w": w_t, "sb": sb_t}
        for i in core_ids
    ]
    res = run_bass_kernel_spmd(nc, in_maps, core_ids)
    return np.concatenate([res.results[i]["out"] for i in core_ids], axis=0)


# revision 7
# speedup vs baseline: 4.3576x; 4.3576x over previous
"""BitConv2dInfer on 8 Trainium2 NeuronCores.

Reference computation (per full input):
    x = clip(x, -1, 1)                       # x [32, 256, 56, 56] f32
    y = conv2d(x, w_q, pad=1)                # w_q [256, 256, 3, 3] ternary
    y = y * s + bias                         # per-out-channel affine
Sharding: data-parallel over batch — each of the 8 cores gets 4 images and
the full (tiny) weights; outputs concatenate over batch with no comms.

Device kernel (per core, per image) — fp8 DoubleRow variant:
  - DMA x[n] in as 2 CIN tiles of [128, 56, 56] f32 (row-chunked for the
    first image so the PE can start before the full image lands)
  - V clamp writes clip(x) straight to fp8e4m3 into a zero-bordered
    [128, 2, 58, 58] tile (DVE fp8-out runs at full rate; the e4m3
    quantization of the clamped activations keeps the end-to-end max
    relative error at 1.58e-2, under the 2e-2 budget — measured against
    the reference on all 32 images)
  - conv as 9 accumulated DoubleRow PE matmuls per (cout_tile, 8-row chunk):
    per tap t: psum += sum_ci w[t,ci].T @ x_win[ci] in ONE matmul
    (lhsT [128, 2, 128] fp8, rhs [128, 2, 8, 56] fp8): DoubleRow contracts
    both cin tiles at once, 2x the bf16 MAC rate (157 TF/s)
  - scalar-engine activation evacuates PSUM with per-partition scale+bias
  - DMA f32 result tiles back out (finely chunked for the last image so the
    tail drains early)

The PE clock gate (HAM) starts at 1.2 GHz and only reaches 2.4 GHz after
~3.4us of sustained activity, so the kernel front-runs dummy matmuls on a
zeroed tile while the first input chunks are in flight.

Weights are host-side transposed to lhsT layout [128 cin, co, (tap, ci), cout]
and cast to fp8e4m3 (exact for ternary values).
"""

import sys

sys.path.insert(0, "/opt/trn_rl_repo")

import ml_dtypes
import numpy as np

import concourse.bass as bass  # noqa: F401  (registers engines)
import concourse.mybir as mybir
import concourse.tile as tile
from concourse import bacc
from concourse.bass_utils import run_bass_kernel_spmd

N, CIN, COUT, H, W = 32, 256, 256, 56, 56
NCORES = 8
NB = N // NCORES          # images per core
HP, WP = H + 2, W + 2     # padded spatial
RG = 8                    # output rows per PSUM chunk (8*56=448 <= 512 f32/bank)
NCH = H // RG             # chunks per image
NCI = CIN // 128          # cin tiles
NCO = COUT // 128         # cout tiles
NTAP = 9
# First-image input chunk schedule, (engine, ci, row0, nrows) in issue order.
# Chunks are sized/placed so the ring stream delivers every chunk before its
# (~1.7us-per-group) deadline. A 9-row first chunk makes matmul group 0
# depend on chunk 0 alone.
N0_CHUNKS = [
    ("g", 0, 0, 9), ("s", 1, 0, 9),
    ("g", 0, 9, 8), ("s", 1, 9, 8),
    ("g", 0, 17, 16), ("s", 1, 17, 8),
    ("s", 1, 25, 8),
    ("g", 0, 33, 12), ("s", 1, 33, 12),
    ("g", 0, 45, 11), ("s", 1, 45, 11),
]
N_WARM_MM = 17            # dummy matmuls to lift the HAM clock gate

F8 = mybir.dt.float8e4
DR = mybir.MatmulPerfMode.DoubleRow

_compiled = {}


def _build():
    nc = bacc.Bacc("TRN2", target_bir_lowering=False, debug=False)
    f32, bf16 = mybir.dt.float32, mybir.dt.bfloat16
    x_d = nc.dram_tensor("x", [NB, CIN, H, W], f32, kind="ExternalInput").ap()
    w_d = nc.dram_tensor(
        "w", [128, NCO, NTAP * NCI, 128], F8, kind="ExternalInput"
    ).ap()
    sb_d = nc.dram_tensor("sb", [128, 2 * NCO], f32, kind="ExternalInput").ap()
    o_d = nc.dram_tensor("out", [NB, COUT, H, W], f32, kind="ExternalOutput").ap()

    clamp = dict(op0=mybir.AluOpType.max, op1=mybir.AluOpType.min)

    with tile.TileContext(nc) as tc:
        with (
            tc.tile_pool(name="const", bufs=1) as cpool,
            tc.tile_pool(name="xs", bufs=4) as xspool,
            tc.tile_pool(name="xsc", bufs=3) as xscpool,
            tc.tile_pool(name="xpad", bufs=2) as xppool,
            tc.tile_pool(name="osb", bufs=3) as opool,
            tc.tile_pool(name="ps", bufs=6, space="PSUM") as pspool,
            tc.tile_pool(name="warmps", bufs=1, space="PSUM") as wpspool,
        ):
            w_sb = cpool.tile([128, NCO, NTAP * NCI, 128], F8, tag="w")
            sb_sb = cpool.tile([128, 2 * NCO], f32, tag="sb")

            # HAM pre-warm (memset on gpsimd so the vector engine's queue
            # stays clear for the border memsets + clamps that gate the
            # first real matmul group).
            warm = cpool.tile([128, RG * W], bf16, tag="warm")
            nc.gpsimd.memset(warm[:], 0.0)
            warm_ps = wpspool.tile([128, RG * W], f32, tag="warmps")
            for _ in range(N_WARM_MM):
                nc.tensor.matmul(
                    out=warm_ps[:], lhsT=warm[:, 0:128], rhs=warm[:],
                    start=True, stop=True,
                )

            def border_memsets(t):
                for ci in range(NCI):
                    nc.vector.memset(t[:, ci, 0:1, :], 0.0)
                    nc.vector.memset(t[:, ci, HP - 1:HP, :], 0.0)
                    nc.vector.memset(t[:, ci, 1:HP - 1, 0:1], 0.0)
                    nc.vector.memset(t[:, ci, 1:HP - 1, WP - 1:WP], 0.0)

            # First image, row-chunked. Critical set for the first matmul
            # group: w piece (taps 0-2 of co=0) + 9 input rows of both ci.
            n0_xp = xppool.tile([128, NCI, HP, WP], F8, tag="xp")
            border_memsets(n0_xp)
            n0_stage = []
            for k, (eng_key, ci, r0, nr) in enumerate(N0_CHUNKS):
                eng = nc.gpsimd if eng_key == "g" else nc.scalar
                xs = xscpool.tile([128, 16, W], f32, tag=f"xsc{ci}")
                eng.dma_start(
                    out=xs[:, 0:nr],
                    in_=x_d[0, ci * 128:(ci + 1) * 128, r0:r0 + nr],
                )
                n0_stage.append((r0, nr, ci, xs))
                if k == 1:
                    # First w piece right after the critical x chunks.
                    nc.sync.dma_start(out=w_sb[:, 0, 0:6], in_=w_d[:, 0, 0:6])
                    nc.sync.dma_start(out=w_sb[:, 0, 6:18], in_=w_d[:, 0, 6:18])
            # Non-critical constants last: they'd steal queue bandwidth from
            # the chunk stream the PE is about to wait on.
            nc.sync.dma_start(out=w_sb[:, 1], in_=w_d[:, 1])
            nc.gpsimd.dma_start(out=sb_sb[:], in_=sb_d)
            for r0, nr, ci, xs in n0_stage:
                nc.vector.tensor_scalar(
                    n0_xp[:, ci, r0 + 1:r0 + nr + 1, 1:W + 1],
                    xs[:, 0:nr], -1.0, 1.0, **clamp,
                )

            # Full-image input DMA, issued one image ahead of its use so the
            # issue (and the transfer) runs behind the previous image's PE
            # work instead of serializing after its PSUM evictions.
            def issue_input(n):
                tiles = []
                for ci, eng in ((0, nc.gpsimd), (1, nc.scalar)):
                    xs = xspool.tile([128, H, W], f32, tag="xs")
                    eng.dma_start(out=xs[:], in_=x_d[n, ci * 128:(ci + 1) * 128])
                    tiles.append(xs)
                return tiles

            pending = None
            for n in range(NB):
                if n == 0:
                    xp = n0_xp
                else:
                    xp = xppool.tile([128, NCI, HP, WP], F8, tag="xp")
                    border_memsets(xp)
                    for ci in range(NCI):
                        nc.vector.tensor_scalar(
                            xp[:, ci, 1:H + 1, 1:W + 1],
                            pending[ci][:], -1.0, 1.0, **clamp,
                        )
                if n + 1 < NB:
                    pending = issue_input(n + 1)
                for co in range(NCO):
                    last_tile = n == NB - 1 and co == NCO - 1
                    osb = opool.tile([128, H, W], f32, tag="osb")
                    if last_tile:
                        # Final output tile: taper the last groups so the
                        # closing ACT + DMA are small and the tail drains fast.
                        groups = [(c * RG, RG) for c in range(NCH - 1)]
                        groups += [(H - 8, 4), (H - 4, 2), (H - 2, 2)]
                    else:
                        groups = [(c * RG, RG) for c in range(NCH)]
                    for g0, gn in groups:
                        ps = pspool.tile([128, RG, W], f32, tag="ps")
                        for t in range(NTAP):
                            kh, kw = divmod(t, 3)
                            nc.tensor.matmul(
                                out=ps[:, 0:gn],
                                lhsT=w_sb[:, co, t * NCI:(t + 1) * NCI],
                                rhs=xp[:, :, g0 + kh:g0 + kh + gn, kw:kw + W],
                                start=(t == 0), stop=(t == NTAP - 1),
                                perf_mode=DR,
                            )
                        nc.scalar.activation(
                            out=osb[:, g0:g0 + gn, :], in_=ps[:, 0:gn],
                            func=mybir.ActivationFunctionType.Identity,
                            bias=sb_sb[:, NCO + co:NCO + co + 1],
                            scale=sb_sb[:, co:co + 1],
                        )
                        if last_tile:
                            nc.sync.dma_start(
                                out=o_d[n, co * 128:(co + 1) * 128, g0:g0 + gn],
                                in_=osb[:, g0:g0 + gn],
                            )
                    dst = o_d[n, co * 128:(co + 1) * 128]
                    if not last_tile:
                        nc.sync.dma_start(out=dst[:, 0:32], in_=osb[:, 0:32])
                        nc.sync.dma_start(out=dst[:, 32:H], in_=osb[:, 32:H])

    nc.compile()
    return nc


def _prep_weights(w_q, s, bias):
    # lhsT layout: [cin_k (128 partitions), co, (tap, ci), cout_j] so that
    # w_t[k, co, t*2+ci, j] = w_q[co*128 + j, ci*128 + k, kh, kw]
    w_t = (
        w_q.astype(np.float32)
        .transpose(2, 3, 1, 0)                 # [kh, kw, CIN, COUT]
        .reshape(NTAP, NCI, 128, NCO, 128)     # [tap, ci, k, co, j]
        .transpose(2, 3, 0, 1, 4)              # [k, co, tap, ci, j]
        .reshape(128, NCO, NTAP * NCI, 128)
        .astype(ml_dtypes.float8_e4m3)
    )
    sb_t = np.concatenate(
        [
            np.ascontiguousarray(s.reshape(NCO, 128).T.astype(np.float32)),
            np.ascontiguousarray(bias.reshape(NCO, 128).T.astype(np.float32)),
        ],
        axis=1,
    )
    return w_t, np.ascontiguousarray(sb_t)


def kernel(x, w_q, s, bias):
    if "nc" not in _compiled:
        _compiled["nc"] = _build()
    nc = _compiled["nc"]

    w_t, sb_t = _prep_weights(w_q, s, bias)
    x = np.ascontiguousarray(x, dtype=np.float32)
    core_ids = list(range(NCORES))
    in_maps = [
        {"x": x[i * NB:(i + 1) * NB], "w": w_t, "sb": sb_t}
        for i in core_ids
    ]
    res = run_bass_kernel_spmd(nc, in_maps, core_ids)
    return np.concatenate([res.results[i]["out"] for i in core_ids], axis=0)
